# revision 10
# baseline (speedup 1.0000x reference)
"""Bass/Trainium2 kernel for nn_DynamicNeuralGraph (gnn_message_passing).

Key reduction: the sequential edge scan h[tgt] += w * h[src] is a linear
operator on h0 along the neuron axis: h_final = M @ h0 with
M = L_{E-1} ... L_0, L_e = I + w_e * e_{tgt} e_{src}^T.  The output is
mean_n h_final[n] = v^T h0 with v^T = (1/N) * ones^T M, and v is computed
by propagating a row vector through the edges in reverse order
(u[src] += w * u[tgt]) -- 4096 scalar ops, done on host.

Since h0[n] = x @ W[n] + b[n]:
    out = x @ W_eff + b_eff,  W_eff = sum_n v_n W[n],  b_eff = v @ b

The only heavy work is the v-weighted reduction of W (N,784,128) =
102.8 MB read once -> memory-bound.  Each of the 8 cores reduces a
98-wide slice of the input dim I=784 (12.85 MB of W per core).

fp32 matmuls on the TRN2 PE run 2 hardware passes, which would make the
reduction PE-bound; instead W and v are decomposed on the host into
fp16 hi + 2048*residual fp16 lo (same total DMA bytes as fp32, ~fp32
combined precision) and the reduction runs four single-pass fp16 matmul
streams with stationary [vh, vl]:
    psum_hi (2, CH) <- [vh,vl]^T Wh   row scales {1, 1/2048}
    psum_lo (2, CH) <- [vh,vl]^T Wl   row scales {1/2048, 1/2048^2}
PSUM->SBUF copies apply the row scales (DVE tensor_scalar for hi, ACT
activation for lo) and the equal-scale flat components stream back to
DRAM per quarter.  The host sums the four component rows per core,
assembles W_eff (784,128), and finishes with the tiny final GEMM
x @ W_eff + b_eff (0.4% of the reference FLOPs).
"""

import os

import numpy as np

N = 256      # neurons
I = 784      # input dim
H = 128      # hidden dim
B = 256      # batch
M_CORES = 8
ISL = I // M_CORES          # 98 i-rows per core
FD = ISL * H                # 12544 flat (i,h) elements per neuron row
HALF = FD // 2              # 6272: W DMA granularity (1.6 MB per stream)
QRT = FD // 4               # 3136: output staging granularity
CH = 448                    # matmul free-dim chunk (fits one PSUM bank)
NCH = FD // CH              # 28 psum chunks total
LO_SCALE = 2048.0           # 2^11, exact in fp32
N_WARMUP = 36               # dummy bf16 matmuls to warm the PE HAM clock

_compiled = None
_last_results = None  # for test harness introspection


def _build():
    import concourse.bacc as bacc
    import concourse.mybir as mybir
    import concourse.tile as tile

    nc = bacc.Bacc(
        "TRN2",
        target_bir_lowering=False,
        debug=False,
        num_devices=M_CORES,
    )
    f32 = mybir.dt.float32
    f16 = mybir.dt.float16
    bf16 = mybir.dt.bfloat16

    wh = nc.dram_tensor("wh", [N, FD], f16, kind="ExternalInput")
    wl = nc.dram_tensor("wl", [N, FD], f16, kind="ExternalInput")
    vhl = nc.dram_tensor("vhl", [128, 4], f16, kind="ExternalInput")
    sc = nc.dram_tensor("sc", [2, 2], f32, kind="ExternalInput")
    oh = nc.dram_tensor("oh", [2, FD], f32, kind="ExternalOutput")
    ol = nc.dram_tensor("ol", [2, FD], f32, kind="ExternalOutput")

    with tile.TileContext(nc) as tc:
        with (
            tc.tile_pool(name="sb", bufs=1) as sb,
            tc.tile_pool(name="wp", bufs=2) as wp,
            tc.tile_pool(name="wf", bufs=2) as wfp,
            tc.tile_pool(name="psh", bufs=3, space="PSUM") as psh,
            tc.tile_pool(name="psl", bufs=3, space="PSUM") as psl,
            tc.tile_pool(name="psw", bufs=1, space="PSUM") as psw,
        ):
            # PE warm-up: keep the HAM clock-gate busy while the first W
            # halves stream in (no data deps -> scheduled immediately).
            junk = sb.tile([128, 640], bf16, tag="junk")
            nc.vector.memset(junk[:], 0.0)
            pwarm = psw.tile([128, 512], f32, tag="warm")
            for _ in range(N_WARMUP):
                nc.tensor.matmul(
                    pwarm[:], junk[:, 512:640], junk[:, 0:512],
                    start=True, stop=True,
                )

            # small loads on the gpsimd SWDGE ring so the sync HWDGE ring
            # streams W from the first cycle
            vtile = sb.tile([128, 4], f16, tag="v")
            nc.gpsimd.dma_start(vtile[:], vhl[:])
            sctile = sb.tile([2, 2], f32, tag="sc")
            nc.gpsimd.dma_start(sctile[:], sc[:])

            halves = []
            for g in range(2):
                gs = slice(g * HALF, (g + 1) * HALF)
                ah = wp.tile([128, HALF], f16, tag="ah", name=f"ah{g}")
                bh = wp.tile([128, HALF], f16, tag="bh", name=f"bh{g}")
                al = wp.tile([128, HALF], f16, tag="al", name=f"al{g}")
                bl = wp.tile([128, HALF], f16, tag="bl", name=f"bl{g}")
                nc.sync.dma_start(ah[:], wh[0:128, gs])
                nc.sync.dma_start(bh[:], wh[128:256, gs])
                nc.sync.dma_start(al[:], wl[0:128, gs])
                nc.sync.dma_start(bl[:], wl[128:256, gs])
                halves.append((ah, bh, al, bl))

            wfh = wfl = None
            for f in range(NCH):
                ah, bh, al, bl = halves[(f * CH) // HALF]
                fs = slice((f * CH) % HALF, (f * CH) % HALF + CH)
                q, qo = (f * CH) // QRT, (f * CH) % QRT
                if qo == 0:
                    wfh = wfp.tile([2, QRT], f32, tag="wfh", name=f"wfh{q}")
                    wfl = wfp.tile([2, QRT], f32, tag="wfl", name=f"wfl{q}")
                ph = psh.tile([2, CH], f32, tag="acch")
                pl = psl.tile([2, CH], f32, tag="accl")
                # alternate PSUM banks between consecutive matmuls (same-
                # bank pairs serialize on array drain) and reuse each
                # stationary for two in a row (halves LDWEIGHTS count)
                nc.tensor.matmul(
                    ph[:], vtile[:, 0:2], ah[:, fs], start=True, stop=False
                )
                nc.tensor.matmul(
                    pl[:], vtile[:, 0:2], al[:, fs], start=True, stop=False
                )
                nc.tensor.matmul(
                    ph[:], vtile[:, 2:4], bh[:, fs], start=False, stop=True
                )
                nc.tensor.matmul(
                    pl[:], vtile[:, 2:4], bl[:, fs], start=False, stop=True
                )
                # scaled PSUM->SBUF copies, split across DVE and ACT
                nc.vector.tensor_scalar_mul(
                    wfh[0:2, qo : qo + CH], ph[:], sctile[0:2, 0:1]
                )
                nc.scalar.activation(
                    wfl[0:2, qo : qo + CH],
                    pl[:],
                    mybir.ActivationFunctionType.Identity,
                    scale=sctile[0:2, 1:2],
                )
                if qo + CH == QRT:
                    # quarter complete: stream results out on the SWDGE
                    # ring (keeps the HWDGE ring free for the W stream)
                    qs = slice(q * QRT, (q + 1) * QRT)
                    nc.gpsimd.dma_start(oh[:, qs], wfh[:])
                    nc.gpsimd.dma_start(ol[:, qs], wfl[:])

    nc.compile()
    return nc


def _compute_v(edge_index, edge_weights):
    src = np.asarray(edge_index[0], dtype=np.int64)
    tgt = np.asarray(edge_index[1], dtype=np.int64)
    ew = np.asarray(edge_weights, dtype=np.float64)
    u = np.ones(N, dtype=np.float64)
    for e in range(ew.shape[0] - 1, -1, -1):
        u[src[e]] += ew[e] * u[tgt[e]]
    return (u / N).astype(np.float32)


def _split_hi_lo(a):
    """a (fp32) -> (hi fp16, lo fp16) with a ~= hi + lo/LO_SCALE."""
    hi = a.astype(np.float16)
    lo = ((a - hi.astype(np.float32)) * LO_SCALE).astype(np.float16)
    return hi, lo


def kernel(x, W, b, edge_index, edge_weights):
    global _compiled, _last_results
    from concourse.bass_utils import run_bass_kernel_spmd

    x = np.asarray(x, dtype=np.float32)
    W = np.asarray(W, dtype=np.float32)
    b = np.asarray(b, dtype=np.float32)

    v = _compute_v(edge_index, edge_weights)
    b_eff = v @ b  # (H,)

    vh, vl = _split_hi_lo(v)
    # columns: [vh chunk0, vl chunk0, vh chunk1, vl chunk1]
    vhl = np.empty((128, 4), dtype=np.float16)
    vhl[:, 0] = vh[0:128]
    vhl[:, 1] = vl[0:128]
    vhl[:, 2] = vh[128:256]
    vhl[:, 3] = vl[128:256]
    s = np.float32(1.0 / LO_SCALE)
    # column 0 = hi-stream row scales, column 1 = lo-stream row scales
    sc = np.array([[1.0, s], [s, s * s]], dtype=np.float32)

    Wh, Wl = _split_hi_lo(W)

    if _compiled is None:
        _compiled = _build()

    in_maps = []
    for c in range(M_CORES):
        isl = slice(c * ISL, (c + 1) * ISL)
        whc = np.ascontiguousarray(Wh[:, isl, :]).reshape(N, FD)
        wlc = np.ascontiguousarray(Wl[:, isl, :]).reshape(N, FD)
        in_maps.append({"wh": whc, "wl": wlc, "vhl": vhl, "sc": sc})

    trace = bool(int(os.environ.get("KERNEL_TRACE", "0")))
    res = run_bass_kernel_spmd(
        _compiled, in_maps, core_ids=list(range(M_CORES)), trace=trace
    )
    _last_results = res

    # gather: per core W_eff slice = sum of the four equal-scale rows
    w_eff = np.empty((I, H), dtype=np.float32)
    for c, r in enumerate(res.results):
        flat = (r["oh"][0] + r["oh"][1] + r["ol"][0] + r["ol"][1])
        w_eff[c * ISL : (c + 1) * ISL, :] = flat.reshape(ISL, H)
    return (x @ w_eff + b_eff[None, :]).astype(np.float32)


# revision 13
# speedup vs baseline: 1.1677x; 1.1677x over previous
"""Bass/Trainium2 kernel for nn_DynamicNeuralGraph (gnn_message_passing).

Key reduction: the sequential edge scan h[tgt] += w * h[src] is a linear
operator on h0 along the neuron axis: h_final = M @ h0 with
M = L_{E-1} ... L_0, L_e = I + w_e * e_{tgt} e_{src}^T.  The output is
mean_n h_final[n] = v^T h0 with v^T = (1/N) * ones^T M, and v is computed
by propagating a row vector through the edges in reverse order
(u[src] += w * u[tgt]) -- 4096 scalar ops, done on host.

Since h0[n] = x @ W[n] + b[n]:
    out = x @ W_eff + b_eff,  W_eff = sum_n v_n W[n],  b_eff = v @ b

The only heavy work is the v-weighted reduction of W (N,784,128) =
102.8 MB read once -> memory-bound.  Each of the 8 cores reduces a
98-wide slice of the input dim I=784 (12.85 MB of W per core).

fp32 matmuls on the TRN2 PE run 2 hardware passes, which would make the
reduction PE-bound; instead W and v are decomposed on the host into
fp16 hi + 2048*residual fp16 lo (same total DMA bytes as fp32, ~fp32
combined precision) and the reduction runs four single-pass fp16 matmul
streams with stationary [vh, vl]:
    psum_hi (2, CH) <- [vh,vl]^T Wh   row scales {1, 1/2048}
    psum_lo (2, CH) <- [vh,vl]^T Wl   row scales {1/2048, 1/2048^2}
PSUM->SBUF copies apply the row scales (DVE tensor_scalar for hi, ACT
activation for lo) and the equal-scale flat components stream back to
DRAM per quarter.  The host sums the four component rows per core,
assembles W_eff (784,128), and finishes with the tiny final GEMM
x @ W_eff + b_eff (0.4% of the reference FLOPs).
"""

import os

import numpy as np

N = 256      # neurons
I = 784      # input dim
H = 128      # hidden dim
B = 256      # batch
M_CORES = 8
ISL = I // M_CORES          # 98 i-rows per core
FD = ISL * H                # 12544 flat (i,h) elements per neuron row
QRT = FD // 4               # 3136: W DMA + output staging granularity
CH = 448                    # matmul free-dim chunk (fits one PSUM bank)
NCH = FD // CH              # 28 psum chunks total
LO_SCALE = 2048.0           # 2^11, exact in fp32
N_WARMUP = 30               # dummy bf16 matmuls to warm the PE HAM clock

_compiled = None
_last_results = None  # for test harness introspection


def _build():
    import concourse.bacc as bacc
    import concourse.mybir as mybir
    import concourse.tile as tile

    nc = bacc.Bacc(
        "TRN2",
        target_bir_lowering=False,
        debug=False,
        num_devices=M_CORES,
    )
    f32 = mybir.dt.float32
    f16 = mybir.dt.float16
    bf16 = mybir.dt.bfloat16

    wh = nc.dram_tensor("wh", [N, FD], f16, kind="ExternalInput")
    wl = nc.dram_tensor("wl", [N, FD], f16, kind="ExternalInput")
    vhl = nc.dram_tensor("vhl", [128, 4], f16, kind="ExternalInput")
    sc = nc.dram_tensor("sc", [2, 2], f32, kind="ExternalInput")
    oh = nc.dram_tensor("oh", [2, FD], f32, kind="ExternalOutput")
    ol = nc.dram_tensor("ol", [2, FD], f32, kind="ExternalOutput")

    with tile.TileContext(nc) as tc:
        with (
            tc.tile_pool(name="sb", bufs=1) as sb,
            tc.tile_pool(name="wp", bufs=2) as wp,
            tc.tile_pool(name="wf", bufs=2) as wfp,
            tc.tile_pool(name="psh", bufs=3, space="PSUM") as psh,
            tc.tile_pool(name="psl", bufs=3, space="PSUM") as psl,
            tc.tile_pool(name="psw", bufs=1, space="PSUM") as psw,
        ):
            # PE warm-up: keep the HAM clock-gate busy while the first W
            # halves stream in (no data deps -> scheduled immediately).
            junk = sb.tile([128, 640], bf16, tag="junk")
            nc.vector.memset(junk[:], 0.0)
            pwarm = psw.tile([128, 512], f32, tag="warm")
            for _ in range(N_WARMUP):
                nc.tensor.matmul(
                    pwarm[:], junk[:, 512:640], junk[:, 0:512],
                    start=True, stop=True,
                )

            # small loads on the gpsimd SWDGE ring so the sync HWDGE ring
            # streams W from the first cycle
            vtile = sb.tile([128, 4], f16, tag="v")
            nc.gpsimd.dma_start(vtile[:], vhl[:])
            sctile = sb.tile([2, 2], f32, tag="sc")
            nc.gpsimd.dma_start(sctile[:], sc[:])

            # quarter-granularity W stream, all four streams interleaved on
            # the one HWDGE ring (FIFO, each DMA gets full ring bandwidth):
            # compute tracks the stream with at most one quarter of slack,
            # so no compute bunches up after the final DMA byte.
            quarters = []
            for g in range(4):
                gs = slice(g * QRT, (g + 1) * QRT)
                ah = wp.tile([128, QRT], f16, tag="ah", name=f"ah{g}")
                bh = wp.tile([128, QRT], f16, tag="bh", name=f"bh{g}")
                al = wp.tile([128, QRT], f16, tag="al", name=f"al{g}")
                bl = wp.tile([128, QRT], f16, tag="bl", name=f"bl{g}")
                nc.sync.dma_start(ah[:], wh[0:128, gs])
                nc.sync.dma_start(bh[:], wh[128:256, gs])
                nc.sync.dma_start(al[:], wl[0:128, gs])
                nc.sync.dma_start(bl[:], wl[128:256, gs])
                quarters.append((ah, bh, al, bl))

            wfh = wfl = None
            for f in range(NCH):
                q, qo = (f * CH) // QRT, (f * CH) % QRT
                ah, bh, al, bl = quarters[q]
                fs = slice(qo, qo + CH)
                if qo == 0:
                    wfh = wfp.tile([2, QRT], f32, tag="wfh", name=f"wfh{q}")
                    wfl = wfp.tile([2, QRT], f32, tag="wfl", name=f"wfl{q}")
                ph = psh.tile([2, CH], f32, tag="acch")
                pl = psl.tile([2, CH], f32, tag="accl")
                # alternate PSUM banks between consecutive matmuls (same-
                # bank pairs serialize on array drain) and reuse each
                # stationary for two in a row (halves LDWEIGHTS count)
                nc.tensor.matmul(
                    ph[:], vtile[:, 0:2], ah[:, fs], start=True, stop=False
                )
                nc.tensor.matmul(
                    pl[:], vtile[:, 0:2], al[:, fs], start=True, stop=False
                )
                nc.tensor.matmul(
                    ph[:], vtile[:, 2:4], bh[:, fs], start=False, stop=True
                )
                nc.tensor.matmul(
                    pl[:], vtile[:, 2:4], bl[:, fs], start=False, stop=True
                )
                # scaled PSUM->SBUF copies; alternate which engine serves
                # which stream so neither stream lags behind the other
                hi_args = (wfh[0:2, qo : qo + CH], ph[:], sctile[0:2, 0:1])
                lo_args = (wfl[0:2, qo : qo + CH], pl[:], sctile[0:2, 1:2])
                dve_args, act_args = (
                    (hi_args, lo_args) if f % 2 == 0 else (lo_args, hi_args)
                )
                nc.vector.tensor_scalar_mul(*dve_args)
                nc.scalar.activation(
                    act_args[0],
                    act_args[1],
                    mybir.ActivationFunctionType.Identity,
                    scale=act_args[2],
                )
                if qo + CH == QRT:
                    # quarter complete: stream results out on the SWDGE
                    # ring (keeps the HWDGE ring free for the W stream)
                    qs = slice(q * QRT, (q + 1) * QRT)
                    nc.gpsimd.dma_start(oh[:, qs], wfh[:])
                    nc.gpsimd.dma_start(ol[:, qs], wfl[:])

    nc.compile()
    return nc


def _compute_v(edge_index, edge_weights):
    src = np.asarray(edge_index[0], dtype=np.int64)
    tgt = np.asarray(edge_index[1], dtype=np.int64)
    ew = np.asarray(edge_weights, dtype=np.float64)
    u = np.ones(N, dtype=np.float64)
    for e in range(ew.shape[0] - 1, -1, -1):
        u[src[e]] += ew[e] * u[tgt[e]]
    return (u / N).astype(np.float32)


def _split_hi_lo(a):
    """a (fp32) -> (hi fp16, lo fp16) with a ~= hi + lo/LO_SCALE."""
    hi = a.astype(np.float16)
    lo = ((a - hi.astype(np.float32)) * LO_SCALE).astype(np.float16)
    return hi, lo


def kernel(x, W, b, edge_index, edge_weights):
    global _compiled, _last_results
    from concourse.bass_utils import run_bass_kernel_spmd

    x = np.asarray(x, dtype=np.float32)
    W = np.asarray(W, dtype=np.float32)
    b = np.asarray(b, dtype=np.float32)

    v = _compute_v(edge_index, edge_weights)
    b_eff = v @ b  # (H,)

    vh, vl = _split_hi_lo(v)
    # columns: [vh chunk0, vl chunk0, vh chunk1, vl chunk1]
    vhl = np.empty((128, 4), dtype=np.float16)
    vhl[:, 0] = vh[0:128]
    vhl[:, 1] = vl[0:128]
    vhl[:, 2] = vh[128:256]
    vhl[:, 3] = vl[128:256]
    s = np.float32(1.0 / LO_SCALE)
    # column 0 = hi-stream row scales, column 1 = lo-stream row scales
    sc = np.array([[1.0, s], [s, s * s]], dtype=np.float32)

    Wh, Wl = _split_hi_lo(W)

    if _compiled is None:
        _compiled = _build()

    in_maps = []
    for c in range(M_CORES):
        isl = slice(c * ISL, (c + 1) * ISL)
        whc = np.ascontiguousarray(Wh[:, isl, :]).reshape(N, FD)
        wlc = np.ascontiguousarray(Wl[:, isl, :]).reshape(N, FD)
        in_maps.append({"wh": whc, "wl": wlc, "vhl": vhl, "sc": sc})

    trace = bool(int(os.environ.get("KERNEL_TRACE", "0")))
    res = run_bass_kernel_spmd(
        _compiled, in_maps, core_ids=list(range(M_CORES)), trace=trace
    )
    _last_results = res

    # gather: per core W_eff slice = sum of the four equal-scale rows
    w_eff = np.empty((I, H), dtype=np.float32)
    for c, r in enumerate(res.results):
        flat = (r["oh"][0] + r["oh"][1] + r["ol"][0] + r["ol"][1])
        w_eff[c * ISL : (c + 1) * ISL, :] = flat.reshape(ISL, H)
    return (x @ w_eff + b_eff[None, :]).astype(np.float32)


# revision 16
# speedup vs baseline: 1.2332x; 1.0561x over previous
"""Bass/Trainium2 kernel for nn_DynamicNeuralGraph (gnn_message_passing).

Key reduction: the sequential edge scan h[tgt] += w * h[src] is a linear
operator on h0 along the neuron axis: h_final = M @ h0 with
M = L_{E-1} ... L_0, L_e = I + w_e * e_{tgt} e_{src}^T.  The output is
mean_n h_final[n] = v^T h0 with v^T = (1/N) * ones^T M, and v is computed
by propagating a row vector through the edges in reverse order
(u[src] += w * u[tgt]) -- 4096 scalar ops, done on host.

Since h0[n] = x @ W[n] + b[n]:
    out = x @ W_eff + b_eff,  W_eff = sum_n v_n W[n],  b_eff = v @ b

The only heavy work is the v-weighted reduction of W (N,784,128) =
102.8 MB read once -> memory-bound.  Each of the 8 cores reduces a
98-wide slice of the input dim I=784 (12.85 MB of W per core).

fp32 matmuls on the TRN2 PE run 2 hardware passes, which would make the
reduction PE-bound; instead W and v are decomposed on the host into
fp16 hi + 2048*residual fp16 lo (same total DMA bytes as fp32, ~fp32
combined precision) and the reduction runs four single-pass fp16 matmul
streams with stationary [vh, vl]:
    psum_hi (2, CH) <- [vh,vl]^T Wh   row scales {1, 1/2048}
    psum_lo (2, CH) <- [vh,vl]^T Wl   row scales {1/2048, 1/2048^2}
PSUM->SBUF copies apply the row scales (DVE tensor_scalar for hi, ACT
activation for lo) and the equal-scale flat components stream back to
DRAM per quarter.  The host sums the four component rows per core,
assembles W_eff (784,128), and finishes with the tiny final GEMM
x @ W_eff + b_eff (0.4% of the reference FLOPs).
"""

import os

import numpy as np

N = 256      # neurons
I = 784      # input dim
H = 128      # hidden dim
B = 256      # batch
M_CORES = 8
ISL = I // M_CORES          # 98 i-rows per core
FD = ISL * H                # 12544 flat (i,h) elements per neuron row
CH = 448                    # matmul free-dim chunk (fits one PSUM bank)
NCH = FD // CH              # 28 psum chunks total
# W DMA / output staging granularity in chunks: big pieces for bandwidth,
# small final pieces so little work bunches up after the last DMA byte
PIECES = [7, 7, 7, 5, 2]
LO_SCALE = 2048.0           # 2^11, exact in fp32
N_WARMUP = 30               # dummy bf16 matmuls to warm the PE HAM clock

_compiled = None
_last_results = None  # for test harness introspection


def _build():
    import concourse.bacc as bacc
    import concourse.mybir as mybir
    import concourse.tile as tile

    nc = bacc.Bacc(
        "TRN2",
        target_bir_lowering=False,
        debug=False,
        num_devices=M_CORES,
    )
    f32 = mybir.dt.float32
    f16 = mybir.dt.float16
    bf16 = mybir.dt.bfloat16

    wh = nc.dram_tensor("wh", [N, FD], f16, kind="ExternalInput")
    wl = nc.dram_tensor("wl", [N, FD], f16, kind="ExternalInput")
    vhl = nc.dram_tensor("vhl", [128, 4], f16, kind="ExternalInput")
    sc = nc.dram_tensor("sc", [2, 2], f32, kind="ExternalInput")
    oh = nc.dram_tensor("oh", [2, FD], f32, kind="ExternalOutput")
    ol = nc.dram_tensor("ol", [2, FD], f32, kind="ExternalOutput")

    with tile.TileContext(nc) as tc:
        with (
            tc.tile_pool(name="sb", bufs=1) as sb,
            tc.tile_pool(name="wp", bufs=2) as wp,
            tc.tile_pool(name="wf", bufs=2) as wfp,
            tc.tile_pool(name="psh", bufs=3, space="PSUM") as psh,
            tc.tile_pool(name="psl", bufs=3, space="PSUM") as psl,
            tc.tile_pool(name="psw", bufs=1, space="PSUM") as psw,
        ):
            # PE warm-up: keep the HAM clock-gate busy while the first W
            # halves stream in (no data deps -> scheduled immediately).
            junk = sb.tile([128, 640], bf16, tag="junk")
            nc.vector.memset(junk[:], 0.0)
            pwarm = psw.tile([128, 512], f32, tag="warm")
            for _ in range(N_WARMUP):
                nc.tensor.matmul(
                    pwarm[:], junk[:, 512:640], junk[:, 0:512],
                    start=True, stop=True,
                )

            # small loads on the gpsimd SWDGE ring so the sync HWDGE ring
            # streams W from the first cycle
            vtile = sb.tile([128, 4], f16, tag="v")
            nc.gpsimd.dma_start(vtile[:], vhl[:])
            sctile = sb.tile([2, 2], f32, tag="sc")
            nc.gpsimd.dma_start(sctile[:], sc[:])

            # piece-granularity W stream, all four streams interleaved on
            # the one HWDGE ring (FIFO, each DMA gets full ring bandwidth):
            # compute tracks the stream with at most one piece of slack,
            # so no compute bunches up after the final DMA byte.
            pieces = []
            piece_of = []
            off = 0
            for g, nch in enumerate(PIECES):
                gs = slice(off * CH, (off + nch) * CH)
                sz = nch * CH
                ah = wp.tile([128, sz], f16, tag="ah", name=f"ah{g}")
                bh = wp.tile([128, sz], f16, tag="bh", name=f"bh{g}")
                al = wp.tile([128, sz], f16, tag="al", name=f"al{g}")
                bl = wp.tile([128, sz], f16, tag="bl", name=f"bl{g}")
                nc.sync.dma_start(ah[:], wh[0:128, gs])
                nc.sync.dma_start(bh[:], wh[128:256, gs])
                nc.sync.dma_start(al[:], wl[0:128, gs])
                nc.sync.dma_start(bl[:], wl[128:256, gs])
                pieces.append((ah, bh, al, bl, off, nch))
                piece_of += [g] * nch
                off += nch

            wfh = wfl = None
            for f in range(NCH):
                q = piece_of[f]
                ah, bh, al, bl, p_off, p_nch = pieces[q]
                qo = (f - p_off) * CH
                fs = slice(qo, qo + CH)
                if qo == 0:
                    wfh = wfp.tile(
                        [2, p_nch * CH], f32, tag="wfh", name=f"wfh{q}"
                    )
                    wfl = wfp.tile(
                        [2, p_nch * CH], f32, tag="wfl", name=f"wfl{q}"
                    )
                ph = psh.tile([2, CH], f32, tag="acch")
                pl = psl.tile([2, CH], f32, tag="accl")
                # alternate PSUM banks between consecutive matmuls (same-
                # bank pairs serialize on array drain) and reuse each
                # stationary for two in a row (halves LDWEIGHTS count)
                nc.tensor.matmul(
                    ph[:], vtile[:, 0:2], ah[:, fs], start=True, stop=False
                )
                nc.tensor.matmul(
                    pl[:], vtile[:, 0:2], al[:, fs], start=True, stop=False
                )
                nc.tensor.matmul(
                    ph[:], vtile[:, 2:4], bh[:, fs], start=False, stop=True
                )
                nc.tensor.matmul(
                    pl[:], vtile[:, 2:4], bl[:, fs], start=False, stop=True
                )
                # scaled PSUM->SBUF copies; alternate which engine serves
                # which stream so neither stream lags behind the other
                hi_args = (wfh[0:2, qo : qo + CH], ph[:], sctile[0:2, 0:1])
                lo_args = (wfl[0:2, qo : qo + CH], pl[:], sctile[0:2, 1:2])
                dve_args, act_args = (
                    (hi_args, lo_args) if f % 2 == 0 else (lo_args, hi_args)
                )
                nc.vector.tensor_scalar_mul(*dve_args)
                nc.scalar.activation(
                    act_args[0],
                    act_args[1],
                    mybir.ActivationFunctionType.Identity,
                    scale=act_args[2],
                )
                if f + 1 - p_off == p_nch:
                    # piece complete: stream results out on the SWDGE
                    # ring (keeps the HWDGE ring free for the W stream)
                    qs = slice(p_off * CH, (p_off + p_nch) * CH)
                    nc.gpsimd.dma_start(oh[:, qs], wfh[:])
                    if q == len(PIECES) - 1:
                        # last piece: the sync ring is idle by now; put the
                        # second output there so the two don't serialize
                        nc.sync.dma_start(ol[:, qs], wfl[:])
                    else:
                        nc.gpsimd.dma_start(ol[:, qs], wfl[:])

    nc.compile()
    return nc


def _compute_v(edge_index, edge_weights):
    src = np.asarray(edge_index[0], dtype=np.int64)
    tgt = np.asarray(edge_index[1], dtype=np.int64)
    ew = np.asarray(edge_weights, dtype=np.float64)
    u = np.ones(N, dtype=np.float64)
    for e in range(ew.shape[0] - 1, -1, -1):
        u[src[e]] += ew[e] * u[tgt[e]]
    return (u / N).astype(np.float32)


def _split_hi_lo(a):
    """a (fp32) -> (hi fp16, lo fp16) with a ~= hi + lo/LO_SCALE."""
    hi = a.astype(np.float16)
    lo = ((a - hi.astype(np.float32)) * LO_SCALE).astype(np.float16)
    return hi, lo


def kernel(x, W, b, edge_index, edge_weights):
    global _compiled, _last_results
    from concourse.bass_utils import run_bass_kernel_spmd

    x = np.asarray(x, dtype=np.float32)
    W = np.asarray(W, dtype=np.float32)
    b = np.asarray(b, dtype=np.float32)

    v = _compute_v(edge_index, edge_weights)
    b_eff = v @ b  # (H,)

    vh, vl = _split_hi_lo(v)
    # columns: [vh chunk0, vl chunk0, vh chunk1, vl chunk1]
    vhl = np.empty((128, 4), dtype=np.float16)
    vhl[:, 0] = vh[0:128]
    vhl[:, 1] = vl[0:128]
    vhl[:, 2] = vh[128:256]
    vhl[:, 3] = vl[128:256]
    s = np.float32(1.0 / LO_SCALE)
    # column 0 = hi-stream row scales, column 1 = lo-stream row scales
    sc = np.array([[1.0, s], [s, s * s]], dtype=np.float32)

    Wh, Wl = _split_hi_lo(W)

    if _compiled is None:
        _compiled = _build()

    in_maps = []
    for c in range(M_CORES):
        isl = slice(c * ISL, (c + 1) * ISL)
        whc = np.ascontiguousarray(Wh[:, isl, :]).reshape(N, FD)
        wlc = np.ascontiguousarray(Wl[:, isl, :]).reshape(N, FD)
        in_maps.append({"wh": whc, "wl": wlc, "vhl": vhl, "sc": sc})

    trace = bool(int(os.environ.get("KERNEL_TRACE", "0")))
    res = run_bass_kernel_spmd(
        _compiled, in_maps, core_ids=list(range(M_CORES)), trace=trace
    )
    _last_results = res

    # gather: per core W_eff slice = sum of the four equal-scale rows
    w_eff = np.empty((I, H), dtype=np.float32)
    for c, r in enumerate(res.results):
        flat = (r["oh"][0] + r["oh"][1] + r["ol"][0] + r["ol"][1])
        w_eff[c * ISL : (c + 1) * ISL, :] = flat.reshape(ISL, H)
    return (x @ w_eff + b_eff[None, :]).astype(np.float32)


# revision 17
# speedup vs baseline: 1.3103x; 1.0625x over previous
"""Bass/Trainium2 kernel for nn_DynamicNeuralGraph (gnn_message_passing).

Key reduction: the sequential edge scan h[tgt] += w * h[src] is a linear
operator on h0 along the neuron axis: h_final = M @ h0 with
M = L_{E-1} ... L_0, L_e = I + w_e * e_{tgt} e_{src}^T.  The output is
mean_n h_final[n] = v^T h0 with v^T = (1/N) * ones^T M, and v is computed
by propagating a row vector through the edges in reverse order
(u[src] += w * u[tgt]) -- 4096 scalar ops, done on host.

Since h0[n] = x @ W[n] + b[n]:
    out = x @ W_eff + b_eff,  W_eff = sum_n v_n W[n],  b_eff = v @ b

The only heavy work is the v-weighted reduction of W (N,784,128) =
102.8 MB read once -> memory-bound.  Each of the 8 cores reduces a
98-wide slice of the input dim I=784 (12.85 MB of W per core).

fp32 matmuls on the TRN2 PE run 2 hardware passes, which would make the
reduction PE-bound; instead W and v are decomposed on the host into
fp16 hi + 2048*residual fp16 lo (same total DMA bytes as fp32, ~fp32
combined precision) and the reduction runs four single-pass fp16 matmul
streams with stationary [vh, vl]:
    psum_hi (2, CH) <- [vh,vl]^T Wh   row scales {1, 1/2048}
    psum_lo (2, CH) <- [vh,vl]^T Wl   row scales {1/2048, 1/2048^2}
PSUM->SBUF copies apply the row scales (DVE tensor_scalar for hi, ACT
activation for lo) and the equal-scale flat components stream back to
DRAM per quarter.  The host sums the four component rows per core,
assembles W_eff (784,128), and finishes with the tiny final GEMM
x @ W_eff + b_eff (0.4% of the reference FLOPs).
"""

import os

import numpy as np

N = 256      # neurons
I = 784      # input dim
H = 128      # hidden dim
B = 256      # batch
M_CORES = 8
ISL = I // M_CORES          # 98 i-rows per core
FD = ISL * H                # 12544 flat (i,h) elements per neuron row
CH = 448                    # matmul free-dim chunk (fits one PSUM bank)
NCH = FD // CH              # 28 psum chunks total
# W DMA / output staging granularity in chunks: big pieces for bandwidth,
# small final pieces so little work bunches up after the last DMA byte
PIECES = [7, 7, 7, 5, 2]
LO_SCALE = 2048.0           # 2^11, exact in fp32 (v residual scale)
W_SCALE = float(2 ** 20)    # W residual scale into fp8e4m3 range
N_WARMUP = 30               # dummy bf16 matmuls to warm the PE HAM clock

_compiled = None
_last_results = None  # for test harness introspection


def _build():
    import concourse.bacc as bacc
    import concourse.mybir as mybir
    import concourse.tile as tile

    nc = bacc.Bacc(
        "TRN2",
        target_bir_lowering=False,
        debug=False,
        num_devices=M_CORES,
    )
    f32 = mybir.dt.float32
    f16 = mybir.dt.float16
    f8 = mybir.dt.float8e4
    bf16 = mybir.dt.bfloat16

    wh = nc.dram_tensor("wh", [N, FD], f16, kind="ExternalInput")
    wl = nc.dram_tensor("wl", [N, FD], f8, kind="ExternalInput")
    vhl = nc.dram_tensor("vhl", [128, 4], f16, kind="ExternalInput")
    sc = nc.dram_tensor("sc", [2, 2], f32, kind="ExternalInput")
    oh = nc.dram_tensor("oh", [2, FD], f32, kind="ExternalOutput")
    ol = nc.dram_tensor("ol", [2, FD], f32, kind="ExternalOutput")

    with tile.TileContext(nc) as tc:
        with (
            tc.tile_pool(name="sb", bufs=1) as sb,
            tc.tile_pool(name="wp", bufs=2) as wp,
            tc.tile_pool(name="wf", bufs=2) as wfp,
            tc.tile_pool(name="psh", bufs=3, space="PSUM") as psh,
            tc.tile_pool(name="psl", bufs=3, space="PSUM") as psl,
            tc.tile_pool(name="psw", bufs=1, space="PSUM") as psw,
        ):
            # PE warm-up: keep the HAM clock-gate busy while the first W
            # halves stream in (no data deps -> scheduled immediately).
            junk = sb.tile([128, 640], bf16, tag="junk")
            nc.vector.memset(junk[:], 0.0)
            pwarm = psw.tile([128, 512], f32, tag="warm")
            for _ in range(N_WARMUP):
                nc.tensor.matmul(
                    pwarm[:], junk[:, 512:640], junk[:, 0:512],
                    start=True, stop=True,
                )

            # small loads on the gpsimd SWDGE ring so the sync HWDGE ring
            # streams W from the first cycle
            vtile = sb.tile([128, 4], f16, tag="v")
            nc.gpsimd.dma_start(vtile[:], vhl[:])
            sctile = sb.tile([2, 2], f32, tag="sc")
            nc.gpsimd.dma_start(sctile[:], sc[:])

            # piece-granularity W stream, all four streams interleaved on
            # the one HWDGE ring (FIFO, each DMA gets full ring bandwidth):
            # compute tracks the stream with at most one piece of slack,
            # so no compute bunches up after the final DMA byte.
            pieces = []
            piece_of = []
            off = 0
            for g, nch in enumerate(PIECES):
                gs = slice(off * CH, (off + nch) * CH)
                sz = nch * CH
                ah = wp.tile([128, sz], f16, tag="ah", name=f"ah{g}")
                bh = wp.tile([128, sz], f16, tag="bh", name=f"bh{g}")
                al = wp.tile([128, sz], f8, tag="al", name=f"al{g}")
                bl = wp.tile([128, sz], f8, tag="bl", name=f"bl{g}")
                nc.sync.dma_start(ah[:], wh[0:128, gs])
                nc.sync.dma_start(bh[:], wh[128:256, gs])
                nc.sync.dma_start(al[:], wl[0:128, gs])
                nc.sync.dma_start(bl[:], wl[128:256, gs])
                pieces.append((ah, bh, al, bl, off, nch))
                piece_of += [g] * nch
                off += nch

            wfh = wfl = None
            for f in range(NCH):
                q = piece_of[f]
                ah, bh, al, bl, p_off, p_nch = pieces[q]
                qo = (f - p_off) * CH
                fs = slice(qo, qo + CH)
                if qo == 0:
                    wfh = wfp.tile(
                        [2, p_nch * CH], f32, tag="wfh", name=f"wfh{q}"
                    )
                    wfl = wfp.tile(
                        [2, p_nch * CH], f32, tag="wfl", name=f"wfl{q}"
                    )
                ph = psh.tile([2, CH], f32, tag="acch")
                pl = psl.tile([2, CH], f32, tag="accl")
                # alternate PSUM banks between consecutive matmuls (same-
                # bank pairs serialize on array drain) and reuse each
                # stationary for two in a row (halves LDWEIGHTS count)
                nc.tensor.matmul(
                    ph[:], vtile[:, 0:2], ah[:, fs], start=True, stop=False
                )
                nc.tensor.matmul(
                    pl[:], vtile[:, 0:2], al[:, fs], start=True, stop=False
                )
                nc.tensor.matmul(
                    ph[:], vtile[:, 2:4], bh[:, fs], start=False, stop=True
                )
                nc.tensor.matmul(
                    pl[:], vtile[:, 2:4], bl[:, fs], start=False, stop=True
                )
                # scaled PSUM->SBUF copies; alternate which engine serves
                # which stream so neither stream lags behind the other
                hi_args = (wfh[0:2, qo : qo + CH], ph[:], sctile[0:2, 0:1])
                lo_args = (wfl[0:2, qo : qo + CH], pl[:], sctile[0:2, 1:2])
                dve_args, act_args = (
                    (hi_args, lo_args) if f % 2 == 0 else (lo_args, hi_args)
                )
                nc.vector.tensor_scalar_mul(*dve_args)
                nc.scalar.activation(
                    act_args[0],
                    act_args[1],
                    mybir.ActivationFunctionType.Identity,
                    scale=act_args[2],
                )
                if f + 1 - p_off == p_nch:
                    # piece complete: stream results out on the SWDGE
                    # ring (keeps the HWDGE ring free for the W stream)
                    qs = slice(p_off * CH, (p_off + p_nch) * CH)
                    nc.gpsimd.dma_start(oh[:, qs], wfh[:])
                    if q == len(PIECES) - 1:
                        # last piece: the sync ring is idle by now; put the
                        # second output there so the two don't serialize
                        nc.sync.dma_start(ol[:, qs], wfl[:])
                    else:
                        nc.gpsimd.dma_start(ol[:, qs], wfl[:])

    nc.compile()
    return nc


def _compute_v(edge_index, edge_weights):
    src = np.asarray(edge_index[0], dtype=np.int64)
    tgt = np.asarray(edge_index[1], dtype=np.int64)
    ew = np.asarray(edge_weights, dtype=np.float64)
    u = np.ones(N, dtype=np.float64)
    for e in range(ew.shape[0] - 1, -1, -1):
        u[src[e]] += ew[e] * u[tgt[e]]
    return (u / N).astype(np.float32)


def _split_hi_lo(a):
    """a (fp32) -> (hi fp16, lo fp16) with a ~= hi + lo/LO_SCALE."""
    hi = a.astype(np.float16)
    lo = ((a - hi.astype(np.float32)) * LO_SCALE).astype(np.float16)
    return hi, lo


def _split_w(a):
    """a (fp32) -> (hi fp16, lo fp8e4m3) with a ~= hi + lo/W_SCALE.

    fp16 hi carries 11 mantissa bits; the fp8 residual adds ~4 more.
    Combined W_eff error lands ~1e-5 relative -- far inside the output
    tolerance -- while cutting the W stream from 4 to 3 bytes/element."""
    import ml_dtypes

    hi = a.astype(np.float16)
    lo = ((a - hi.astype(np.float32)) * W_SCALE).astype(ml_dtypes.float8_e4m3)
    return hi, lo


def kernel(x, W, b, edge_index, edge_weights):
    global _compiled, _last_results
    from concourse.bass_utils import run_bass_kernel_spmd

    x = np.asarray(x, dtype=np.float32)
    W = np.asarray(W, dtype=np.float32)
    b = np.asarray(b, dtype=np.float32)

    v = _compute_v(edge_index, edge_weights)
    b_eff = v @ b  # (H,)

    vh, vl = _split_hi_lo(v)
    # columns: [vh chunk0, vl chunk0, vh chunk1, vl chunk1]
    vhl = np.empty((128, 4), dtype=np.float16)
    vhl[:, 0] = vh[0:128]
    vhl[:, 1] = vl[0:128]
    vhl[:, 2] = vh[128:256]
    vhl[:, 3] = vl[128:256]
    s = np.float32(1.0 / LO_SCALE)
    sw = np.float32(1.0 / W_SCALE)
    # column 0 = hi-stream row scales, column 1 = lo-stream row scales
    sc = np.array([[1.0, sw], [s, s * sw]], dtype=np.float32)

    Wh, Wl = _split_w(W)

    if _compiled is None:
        _compiled = _build()

    in_maps = []
    for c in range(M_CORES):
        isl = slice(c * ISL, (c + 1) * ISL)
        whc = np.ascontiguousarray(Wh[:, isl, :]).reshape(N, FD)
        wlc = np.ascontiguousarray(Wl[:, isl, :]).reshape(N, FD)
        in_maps.append({"wh": whc, "wl": wlc, "vhl": vhl, "sc": sc})

    trace = bool(int(os.environ.get("KERNEL_TRACE", "0")))
    res = run_bass_kernel_spmd(
        _compiled, in_maps, core_ids=list(range(M_CORES)), trace=trace
    )
    _last_results = res

    # gather: per core W_eff slice = sum of the four equal-scale rows
    w_eff = np.empty((I, H), dtype=np.float32)
    for c, r in enumerate(res.results):
        flat = (r["oh"][0] + r["oh"][1] + r["ol"][0] + r["ol"][1])
        w_eff[c * ISL : (c + 1) * ISL, :] = flat.reshape(ISL, H)
    return (x @ w_eff + b_eff[None, :]).astype(np.float32)
